# revision 13
# baseline (speedup 1.0000x reference)
"""HRM (hierarchical GRU) Bass kernel for Trainium2, 8-core data parallel.

Layout strategy: activations are kept feature-major ([feature, batch] on
SBUF) so every matmul streams batch columns through the PE array.  The
recurrence exploits:
  - x_embed @ Wx.T            computed once per batch tile ("ax")
  - z_h @ Wh.T                recomputed once per segment ("a_comb")
  - r/z gate weights of w_ih and w_hh folded into one matrix (same input)
  - all matmuls in float32r (full-rate PE, ~1e-4 rounding)
  - "+a_comb" folded into PSUM accumulation via identity matmul
"""

import numpy as np

import concourse.bass as bass
import concourse.mybir as mybir
import concourse.tile as tile
import bass_rust as br
from concourse.alu_op_type import AluOpType
from concourse.masks import make_identity
from concourse.bass_utils import run_bass_kernel_spmd

F32 = mybir.dt.float32
F32R = mybir.dt.float32r
AF = mybir.ActivationFunctionType

B, IN_DIM = 65536, 512
EMB = HL = HH = 256
N_CORES = 8
BC = B // N_CORES          # rows per core
TN = 512                   # batch tile (free-dim) size
LN_EPS = 1e-5


# --------------------------------------------------------------------------
# host-side weight prep
# --------------------------------------------------------------------------

def _prep(inp):
    low_w_ih = inp["low_w_ih"]; low_w_hh = inp["low_w_hh"]
    high_w_ih = inp["high_w_ih"]; high_w_hh = inp["high_w_hh"]
    out_w = inp["out_w"]; ln_g = inp["ln_g"]; ln_b = inp["ln_b"]

    wx = low_w_ih[:, 0:256]; wh = low_w_ih[:, 256:512]; wl = low_w_ih[:, 512:768]
    a_hi = high_w_ih[:, 0:256]; b_hi = high_w_ih[:, 256:512]

    pieces = {}
    pieces["pw"] = (inp["proj_w"].T, 4)                     # [512,256] 4 k-chunks
    pieces["wx"] = (wx.T, 2)                                 # [256,768]
    pieces["wh"] = (wh.T, 2)
    pieces["wlrz"] = ((wl[0:512] + low_w_hh[0:512]).T, 2)    # [256,512]
    pieces["wln"] = (wl[512:768].T, 2)                       # [256,256]
    pieces["whhn"] = (low_w_hh[512:768].T, 2)
    pieces["aT"] = (a_hi.T, 2)                               # [256,768]
    pieces["brz"] = ((b_hi[0:512] + high_w_hh[0:512]).T, 2)  # [256,512]
    pieces["bn"] = (b_hi[512:768].T, 2)
    pieces["hhhn"] = (high_w_hh[512:768].T, 2)
    wg = out_w * ln_g[None, :]                               # [2,256]
    pieces["wg"] = (wg.T, 2)                                 # [256,2]

    cols = []
    offs = {}
    pos = 0
    for name, (mat, kchunks) in pieces.items():
        K, M = mat.shape
        assert K == kchunks * 128
        offs[name] = (pos, M)
        for k in range(kchunks):
            cols.append(mat[k * 128:(k + 1) * 128, :])
        pos += kchunks * M
    # identity block for I-add matmuls
    offs["ident"] = (pos, 128)
    cols.append(np.eye(128, dtype=np.float32)); pos += 128
    # ones/256 columns for LN mean reductions (2 k-chunks of [128,1])
    offs["ones"] = (pos, 1)
    cols.append(np.full((128, 1), 1.0 / 256.0, np.float32))
    cols.append(np.full((128, 1), 1.0 / 256.0, np.float32)); pos += 2
    # [1,2] ones row for partition broadcast (row0 only matters)
    offs["ones2"] = (pos, 2)
    cols.append(np.ones((128, 2), np.float32)); pos += 2
    wcat = np.concatenate(cols, axis=1).astype(np.float32)

    # biases -------------------------------------------------------------
    b_comb_low = inp["low_b_ih"] + np.concatenate(
        [inp["low_b_hh"][0:512], np.zeros(256, np.float32)])
    b_comb_high = inp["high_b_ih"] + np.concatenate(
        [inp["high_b_hh"][0:512], np.zeros(256, np.float32)])
    s1 = wg.sum(axis=1)                                      # [2]
    c0 = out_w @ ln_b + inp["out_b"]                         # [2]

    bias = np.zeros((128, 21), np.float32)
    for c in range(6):
        bias[:, c] = b_comb_low[c * 128:(c + 1) * 128]
    for c in range(2):
        bias[:, 6 + c] = inp["low_b_hh"][512 + c * 128: 512 + (c + 1) * 128]
    for c in range(2):
        bias[:, 8 + c] = b_comb_high[c * 128:(c + 1) * 128]
    for c in range(2):
        bias[:, 10 + c] = -b_comb_high[256 + c * 128: 256 + (c + 1) * 128]
    for c in range(2):
        bias[:, 12 + c] = b_comb_high[512 + c * 128: 512 + (c + 1) * 128]
    for c in range(2):
        bias[:, 14 + c] = inp["high_b_hh"][512 + c * 128: 512 + (c + 1) * 128]
    for c in range(2):
        bias[:, 16 + c] = inp["proj_b"][c * 128:(c + 1) * 128]
    bias[0, 18] = LN_EPS
    bias[0:2, 19] = c0
    bias[0:2, 20] = -s1
    return wcat, bias, offs


# --------------------------------------------------------------------------
# device module
# --------------------------------------------------------------------------

def _split_excess_waits(nc, max_waits=1):
    for bb in nc.main_func.blocks:
        nl = []
        for ins in bb.instructions:
            si = ins.sync_info
            if si is not None and si.on_wait and len(si.on_wait) > max_waits:
                w = list(si.on_wait)
                ex, keep = w[max_waits:], w[:max_waits]
                for k, ww in enumerate(ex):
                    stub = mybir.InstEventSemaphore(
                        name=f"{ins.name}-ws{k}", engine=ins.engine,
                        ins=[], outs=[],
                        sync_info=br.SyncInfo(on_wait=[ww], on_update=[]))
                    nc.register_instruction(stub)
                    nl.append(stub)
                ins.sync_info = br.SyncInfo(on_wait=keep,
                                            on_update=list(si.on_update))
            nl.append(ins)
        bb.instructions[:] = nl


def build_module(wcat_cols, n_tiles):
    nc = bass.Bass()
    x_d = nc.declare_dram_parameter("x", [BC, IN_DIM], F32, isOutput=False)
    w_d = nc.declare_dram_parameter("wcat", [128, wcat_cols], F32, isOutput=False)
    bias_d = nc.declare_dram_parameter("bias", [128, 21], F32, isOutput=False)
    zh_d = nc.declare_dram_parameter("zh_o", [BC, HH], F32, isOutput=True)
    zl_d = nc.declare_dram_parameter("zl_o", [BC, HL], F32, isOutput=True)
    lg_d = nc.declare_dram_parameter("lg_o", [BC, 2], F32, isOutput=True)
    return nc, (x_d, w_d, bias_d, zh_d, zl_d, lg_d)


def emit_kernel(nc, params, offs, n_tiles):
    x_d, w_d, bias_d, zh_d, zl_d, lg_d = params
    wcat_cols = w_d.shape[1]

    from contextlib import ExitStack
    ctx = ExitStack()
    tc = ctx.enter_context(tile.TileContext(nc))
    wpool = ctx.enter_context(tc.tile_pool(name="w", bufs=1))
    apool = ctx.enter_context(tc.tile_pool(name="act", bufs=1))
    pspool = ctx.enter_context(tc.tile_pool(name="ps", bufs=1, space="PSUM"))

    # ---- weights: staged DMA + cast to f32r --------------------------------
    wr = wpool.tile([128, wcat_cols], F32R, tag="wr", name="wr")
    nchunk = 8
    cw = ((wcat_cols + nchunk - 1) // nchunk + 3) & ~3
    for i in range(nchunk):
        c0 = i * cw
        c1 = min(wcat_cols, c0 + cw)
        if c0 >= c1:
            break
        wtmp = wpool.tile([128, cw], F32, tag="wtmp", name=f"wtmp{i}", bufs=1)
        nc.sync.dma_start(out=wtmp[:, 0:c1 - c0], in_=w_d[:, c0:c1])
        eng = nc.vector if i % 2 == 0 else nc.gpsimd
        eng.tensor_copy(wr[:, c0:c1], wtmp[:, 0:c1 - c0])
    bias_sb = wpool.tile([128, 21], F32, tag="bias", name="bias")
    nc.sync.dma_start(out=bias_sb[:], in_=bias_d[:, :])
    ident = wpool.tile([128, 128], F32, tag="ident", name="ident")
    make_identity(nc, ident[:])

    def W(name, k, m0, mw=128):
        off, M = offs[name]
        return wr[:, off + k * M + m0: off + k * M + m0 + mw]

    identr = wr[:, offs["ident"][0]: offs["ident"][0] + 128]

    _BUFS = {"xnat": 2, "xT": 1, "xe": 2, "ax": 1, "ac": 1, "zl": 2, "zh": 2,
             "r": 2, "zc": 2, "t": 2, "u": 2, "n": 2, "d": 1, "e": 1,
             "stg": 1, "zsq": 1}
    _seq = [0]

    def PS(tag, shape=(128, TN), dt=F32):
        _seq[0] += 1
        return pspool.tile(list(shape), dt, tag=tag, name=f"{tag}_{_seq[0]}",
                           bufs=1)

    def act(t, shape=(128, TN), dt=F32):
        _seq[0] += 1
        base = t.rstrip("0123456789")
        return apool.tile(list(shape), dt, tag=t, name=f"{t}_{_seq[0]}",
                          bufs=_BUFS.get(base, 1))

    bias_ap = lambda c: bias_sb[:, c:c + 1]

    # ---- per-step bodies ---------------------------------------------------
    def low_step(zl, acomb):
        """one low-GRU step; zl: [2] f32r tiles, acomb: [6] f32r tiles."""
        rz_ps = []
        for m in range(4):
            ps = PS(f"P{m}")
            nc.tensor.matmul(ps[:], W("wlrz", 0, m * 128), zl[0][:], start=True, stop=False)
            nc.tensor.matmul(ps[:], W("wlrz", 1, m * 128), zl[1][:], start=False, stop=False)
            nc.tensor.matmul(ps[:], identr, acomb[m][:], start=False, stop=True)
            rz_ps.append(ps)
        r = []
        zc = []
        for c in range(2):
            rt = act(f"r{c}")
            nc.scalar.activation(out=rt[:], in_=rz_ps[c][:], func=AF.Sigmoid)
            r.append(rt)
        for c in range(2):
            zt = act(f"zc{c}")
            nc.scalar.activation(out=zt[:], in_=rz_ps[2 + c][:], func=AF.Sigmoid, scale=-1.0)
            zc.append(zt)
        gin_ps = []
        for m in range(2):
            ps = PS(f"P{4 + m}")
            nc.tensor.matmul(ps[:], W("wln", 0, m * 128), zl[0][:], start=True, stop=False)
            nc.tensor.matmul(ps[:], W("wln", 1, m * 128), zl[1][:], start=False, stop=False)
            nc.tensor.matmul(ps[:], identr, acomb[4 + m][:], start=False, stop=True)
            gin_ps.append(ps)
        ghn_ps = []
        for m in range(2):
            ps = PS(f"P{6 + m}")
            nc.tensor.matmul(ps[:], W("whhn", 0, m * 128), zl[0][:], start=True, stop=False)
            nc.tensor.matmul(ps[:], W("whhn", 1, m * 128), zl[1][:], start=False, stop=True)
            ghn_ps.append(ps)
        zl_new = []
        for c in range(2):
            tt = act(f"t{c}")
            nc.vector.scalar_tensor_tensor(out=tt[:], in0=ghn_ps[c][:], scalar=bias_ap(6 + c),
                                           in1=r[c][:], op0=AluOpType.add, op1=AluOpType.mult)
            uu = act(f"u{c}")
            nc.vector.tensor_tensor(out=uu[:], in0=tt[:], in1=gin_ps[c][:], op=AluOpType.add)
            nn = act(f"n{c}")
            nc.scalar.activation(out=nn[:], in_=uu[:], func=AF.Tanh)
            dd = act(f"d{c}")
            nc.gpsimd.tensor_tensor(out=dd[:], in0=nn[:], in1=zl[c][:], op=AluOpType.subtract)
            ee = act(f"e{c}")
            nc.gpsimd.tensor_tensor(out=ee[:], in0=zc[c][:], in1=dd[:], op=AluOpType.mult)
            zn = act(f"zl{c}", dt=F32R)
            nc.vector.tensor_tensor(out=zn[:], in0=ee[:], in1=zl[c][:], op=AluOpType.add)
            zl_new.append(zn)
        return zl_new

    def step0(ax):
        """first low step: z_l = z_h = 0, gates come from ax alone."""
        zl1 = []
        r = []
        zc = []
        for c in range(2):
            rt = act(f"r{c}")
            nc.scalar.activation(out=rt[:], in_=ax[c][:], func=AF.Sigmoid)
            r.append(rt)
        for c in range(2):
            zt = act(f"zc{c}")
            nc.scalar.activation(out=zt[:], in_=ax[2 + c][:], func=AF.Sigmoid, scale=-1.0)
            zc.append(zt)
        for c in range(2):
            tt = act(f"t{c}")
            nc.vector.tensor_scalar_mul(tt[:], r[c][:], bias_ap(6 + c))
            uu = act(f"u{c}")
            nc.vector.tensor_tensor(out=uu[:], in0=tt[:], in1=ax[4 + c][:], op=AluOpType.add)
            nn = act(f"n{c}")
            nc.scalar.activation(out=nn[:], in_=uu[:], func=AF.Tanh)
            zn = act(f"zl{c}", dt=F32R)
            nc.vector.tensor_tensor(out=zn[:], in0=zc[c][:], in1=nn[:], op=AluOpType.mult)
            zl1.append(zn)
        return zl1

    def high_step(zl, zh):
        """one high-GRU step; zh may be None (first high step, z_h = 0)."""
        rz_ps = []
        for m in range(4):
            ps = PS(f"P{m}")
            nc.tensor.matmul(ps[:], W("aT", 0, m * 128), zl[0][:], start=True, stop=False)
            nc.tensor.matmul(ps[:], W("aT", 1, m * 128), zl[1][:], start=False, stop=(zh is None))
            if zh is not None:
                nc.tensor.matmul(ps[:], W("brz", 0, m * 128), zh[0][:], start=False, stop=False)
                nc.tensor.matmul(ps[:], W("brz", 1, m * 128), zh[1][:], start=False, stop=True)
            rz_ps.append(ps)
        r = []
        zc = []
        for c in range(2):
            rt = act(f"r{c}")
            nc.scalar.activation(out=rt[:], in_=rz_ps[c][:], func=AF.Sigmoid, bias=bias_ap(8 + c))
            r.append(rt)
        for c in range(2):
            zt = act(f"zc{c}")
            nc.scalar.activation(out=zt[:], in_=rz_ps[2 + c][:], func=AF.Sigmoid,
                                 scale=-1.0, bias=bias_ap(10 + c))
            zc.append(zt)
        gin_ps = []
        for m in range(2):
            ps = PS(f"P{4 + m}")
            nc.tensor.matmul(ps[:], W("aT", 0, 512 + m * 128), zl[0][:], start=True, stop=False)
            nc.tensor.matmul(ps[:], W("aT", 1, 512 + m * 128), zl[1][:], start=False, stop=(zh is None))
            if zh is not None:
                nc.tensor.matmul(ps[:], W("bn", 0, m * 128), zh[0][:], start=False, stop=False)
                nc.tensor.matmul(ps[:], W("bn", 1, m * 128), zh[1][:], start=False, stop=True)
            gin_ps.append(ps)
        ghn_ps = []
        if zh is not None:
            for m in range(2):
                ps = PS(f"P{6 + m}")
                nc.tensor.matmul(ps[:], W("hhhn", 0, m * 128), zh[0][:], start=True, stop=False)
                nc.tensor.matmul(ps[:], W("hhhn", 1, m * 128), zh[1][:], start=False, stop=True)
                ghn_ps.append(ps)
        zh_new = []
        for c in range(2):
            tt = act(f"t{c}")
            if zh is not None:
                nc.vector.scalar_tensor_tensor(out=tt[:], in0=ghn_ps[c][:], scalar=bias_ap(14 + c),
                                               in1=r[c][:], op0=AluOpType.add, op1=AluOpType.mult)
            else:
                nc.vector.tensor_scalar_mul(tt[:], r[c][:], bias_ap(14 + c))
            uu = act(f"u{c}")
            nc.vector.tensor_tensor(out=uu[:], in0=tt[:], in1=gin_ps[c][:], op=AluOpType.add)
            nn = act(f"n{c}")
            nc.scalar.activation(out=nn[:], in_=uu[:], func=AF.Tanh, bias=bias_ap(12 + c))
            zn = act(f"zh{c}", dt=F32R)
            if zh is not None:
                dd = act(f"d{c}")
                nc.gpsimd.tensor_tensor(out=dd[:], in0=nn[:], in1=zh[c][:], op=AluOpType.subtract)
                ee = act(f"e{c}")
                nc.gpsimd.tensor_tensor(out=ee[:], in0=zc[c][:], in1=dd[:], op=AluOpType.mult)
                nc.vector.tensor_tensor(out=zn[:], in0=ee[:], in1=zh[c][:], op=AluOpType.add)
            else:
                nc.vector.tensor_tensor(out=zn[:], in0=zc[c][:], in1=nn[:], op=AluOpType.mult)
            zh_new.append(zn)
        return zh_new

    # ---- main tile loop ----------------------------------------------------
    for t in range(n_tiles):
        r0 = t * TN
        # input DMA: 4 natural tiles [128 batch rows, 512 features]
        xnat = []
        for c in range(4):
            xt = act(f"xnat{c}")
            nc.sync.dma_start(out=xt[:], in_=x_d[r0 + c * 128: r0 + (c + 1) * 128, :])
            xnat.append(xt)
        # transpose x -> feature-major xT [4 x (128 feat, 512 batch)]
        xT = [act(f"xT{fb}", dt=F32R) for fb in range(4)]
        for fb in range(4):
            for cb in range(4):
                tp = PS(f"P{(fb * 4 + cb) % 8}", (128, 128))
                nc.tensor.transpose(tp[:], xnat[cb][:, fb * 128:(fb + 1) * 128], ident[:])
                nc.scalar.activation(out=xT[fb][:, cb * 128:(cb + 1) * 128], in_=tp[:],
                                     func=AF.Identity)
        # projection + exact GELU -> x_embed [2 x (128, TN)] f32r
        xe = []
        for m in range(2):
            ps = PS(f"P{m}")
            for k in range(4):
                nc.tensor.matmul(ps[:], W("pw", k, m * 128), xT[k][:],
                                 start=(k == 0), stop=(k == 3))
            xem = act(f"xe{m}", dt=F32R)
            nc.scalar.activation(out=xem[:], in_=ps[:], func=AF.Gelu, bias=bias_ap(16 + m))
            xe.append(xem)
        # ax = x_embed @ Wx.T + b_comb_low  [6 x (128, TN)] f32r
        ax = []
        for m in range(6):
            ps = PS(f"P{m % 6}")
            nc.tensor.matmul(ps[:], W("wx", 0, m * 128), xe[0][:], start=True, stop=False)
            nc.tensor.matmul(ps[:], W("wx", 1, m * 128), xe[1][:], start=False, stop=True)
            am = act(f"ax{m}", dt=F32R)
            nc.scalar.activation(out=am[:], in_=ps[:], func=AF.Identity, bias=bias_ap(m))
            ax.append(am)

        # ---- recurrence ----
        zl = step0(ax)
        for _ in range(3):
            zl = low_step(zl, ax)
        zh = high_step(zl, None)

        for seg in range(2):
            acomb = []
            for m in range(6):
                ps = PS(f"P{m % 6}")
                nc.tensor.matmul(ps[:], W("wh", 0, m * 128), zh[0][:], start=True, stop=False)
                nc.tensor.matmul(ps[:], W("wh", 1, m * 128), zh[1][:], start=False, stop=False)
                nc.tensor.matmul(ps[:], identr, ax[m][:], start=False, stop=True)
                am = act(f"ac{m}", dt=F32R)
                nc.scalar.activation(out=am[:], in_=ps[:], func=AF.Identity)
                acomb.append(am)
            for _ in range(4):
                zl = low_step(zl, acomb)
            zh = high_step(zl, zh)

        # ---- output head: LayerNorm + tiny linear ----
        zsq = []
        for c in range(2):
            zq = act(f"zsq{c}", dt=F32R)
            nc.scalar.activation(out=zq[:], in_=zh[c][:], func=AF.Square)
            zsq.append(zq)
        mu_ps = PS("P0", (1, TN))
        nc.tensor.matmul(mu_ps[:], W("ones", 0, 0, 1), zh[0][:], start=True, stop=False)
        nc.tensor.matmul(mu_ps[:], W("ones", 1, 0, 1), zh[1][:], start=False, stop=True)
        ex2_ps = PS("P1", (1, TN))
        nc.tensor.matmul(ex2_ps[:], W("ones", 0, 0, 1), zsq[0][:], start=True, stop=False)
        nc.tensor.matmul(ex2_ps[:], W("ones", 1, 0, 1), zsq[1][:], start=False, stop=True)
        wgz_ps = PS("P2", (2, TN))
        nc.tensor.matmul(wgz_ps[:], W("wg", 0, 0, 2), zh[0][:], start=True, stop=False)
        nc.tensor.matmul(wgz_ps[:], W("wg", 1, 0, 2), zh[1][:], start=False, stop=True)
        mu_sb = act("mu", (1, TN), F32R)
        nc.scalar.activation(out=mu_sb[:], in_=mu_ps[:], func=AF.Identity)
        musq = act("musq", (1, TN))
        nc.scalar.activation(out=musq[:], in_=mu_ps[:], func=AF.Square)
        var = act("var", (1, TN))
        nc.vector.tensor_tensor(out=var[:], in0=ex2_ps[:], in1=musq[:], op=AluOpType.subtract)
        sd = act("sd", (1, TN))
        nc.scalar.activation(out=sd[:], in_=var[:], func=AF.Sqrt, bias=bias_sb[0:1, 18:19])
        rstd = act("rstd", (1, TN), F32R)
        with nc.allow_low_precision(reason="f32r is fp32-width storage"):
            nc.vector.reciprocal(rstd[:], sd[:])
        v = act("v", (1, TN), F32R)
        nc.vector.tensor_tensor(out=v[:], in0=mu_sb[:], in1=rstd[:], op=AluOpType.mult)
        rb2_ps = PS("P3", (2, TN))
        nc.tensor.matmul(rb2_ps[:], wr[0:1, offs["ones2"][0]: offs["ones2"][0] + 2],
                         rstd[:], start=True, stop=True)
        v2_ps = PS("P4", (2, TN))
        nc.tensor.matmul(v2_ps[:], wr[0:1, offs["ones2"][0]: offs["ones2"][0] + 2],
                         v[:], start=True, stop=True)
        rb2 = act("rb2", (2, TN))
        nc.scalar.activation(out=rb2[:], in_=rb2_ps[:], func=AF.Identity)
        u2 = act("u2", (2, TN))
        nc.vector.tensor_tensor(out=u2[:], in0=wgz_ps[:], in1=rb2[:], op=AluOpType.mult)
        w2 = act("w2", (2, TN))
        nc.vector.scalar_tensor_tensor(out=w2[:], in0=v2_ps[:], scalar=bias_sb[0:2, 20:21],
                                       in1=u2[:], op0=AluOpType.mult, op1=AluOpType.add)
        lg = act("lg", (2, TN))
        nc.vector.tensor_scalar_add(lg[:], w2[:], bias_sb[0:2, 19:20])
        for j in range(2):
            nc.sync.dma_start(out=lg_d[r0:r0 + TN, j:j + 1].rearrange("a b -> b a"),
                              in_=lg[j:j + 1, :])

        # ---- transpose states back to batch-major and store ----
        for si, (state, dram) in enumerate([(zh, zh_d), (zl, zl_d)]):
            stg = [act(f"stg{si}{cb}", (128, 256)) for cb in range(4)]
            for f in range(2):
                for cb in range(4):
                    tp = PS(f"P{(f * 4 + cb) % 8}", (128, 128), F32R)
                    nc.tensor.transpose(tp[:], state[f][:, cb * 128:(cb + 1) * 128], identr)
                    nc.scalar.activation(out=stg[cb][:, f * 128:(f + 1) * 128],
                                         in_=tp[:], func=AF.Identity)
            for cb in range(4):
                nc.sync.dma_start(out=dram[r0 + cb * 128: r0 + (cb + 1) * 128, :],
                                  in_=stg[cb][:])

    ctx.close()
    return nc


def build(n_tiles=BC // TN):
    # wcat column count must match _prep; compute from a dummy
    dummy = {k: np.zeros(s, np.float32) for k, s in [
        ("proj_w", (256, 512)), ("proj_b", (256,)),
        ("low_w_ih", (768, 768)), ("low_w_hh", (768, 256)),
        ("low_b_ih", (768,)), ("low_b_hh", (768,)),
        ("high_w_ih", (768, 512)), ("high_w_hh", (768, 256)),
        ("high_b_ih", (768,)), ("high_b_hh", (768,)),
        ("ln_g", (256,)), ("ln_b", (256,)), ("out_w", (2, 256)), ("out_b", (2,))]}
    wcat, _, offs = _prep(dummy)
    nc, params = build_module(wcat.shape[1], n_tiles)
    emit_kernel(nc, params, offs, n_tiles)
    _split_excess_waits(nc)
    return nc


_CACHED = {}


def kernel(trace=False, n_tiles=BC // TN, **inputs):
    inputs = {k: np.asarray(v, np.float32) for k, v in inputs.items()}
    wcat, bias, offs = _prep(inputs)
    key = n_tiles
    if key not in _CACHED:
        nc, params = build_module(wcat.shape[1], n_tiles)
        emit_kernel(nc, params, offs, n_tiles)
        _split_excess_waits(nc)
        _CACHED[key] = nc
    nc = _CACHED[key]

    x = inputs["x"]
    in_maps = []
    for c in range(N_CORES):
        in_maps.append({
            "x": np.ascontiguousarray(x[c * BC:(c + 1) * BC]),
            "wcat": wcat, "bias": bias,
        })
    res = run_bass_kernel_spmd(nc, in_maps, core_ids=list(range(N_CORES)),
                               trace=trace)
    rows = n_tiles * TN
    zh = np.empty((B, HH), np.float32)
    zl = np.empty((B, HL), np.float32)
    lg = np.empty((B, 2), np.float32)
    for c in range(N_CORES):
        zh[c * BC: c * BC + rows] = res.results[c]["zh_o"][:rows]
        zl[c * BC: c * BC + rows] = res.results[c]["zl_o"][:rows]
        lg[c * BC: c * BC + rows] = res.results[c]["lg_o"][:rows]
    kernel.last_results = res
    return zh, zl, lg


# revision 15
# speedup vs baseline: 141.5269x; 141.5269x over previous
"""HRM (hierarchical GRU) Bass kernel for Trainium2, 8-core data parallel.

Layout strategy: activations are kept feature-major ([feature, batch] on
SBUF) so every matmul streams batch columns through the PE array.  The
recurrence exploits:
  - x_embed @ Wx.T            computed once per batch tile ("ax")
  - z_h @ Wh.T                recomputed once per segment ("a_comb")
  - r/z gate weights of w_ih and w_hh folded into one matrix (same input)
  - all matmuls in float32r (full-rate PE, ~1e-4 rounding)
  - "+a_comb" folded into PSUM accumulation via identity matmul
"""

import numpy as np

import concourse.bass as bass
import concourse.mybir as mybir
import concourse.tile as tile
import bass_rust as br
from concourse.alu_op_type import AluOpType
from concourse.masks import make_identity
from concourse.bass_utils import run_bass_kernel_spmd

F32 = mybir.dt.float32
F32R = mybir.dt.float32r
AF = mybir.ActivationFunctionType

B, IN_DIM = 65536, 512
EMB = HL = HH = 256
N_CORES = 8
BC = B // N_CORES          # rows per core
TN = 512                   # batch tile (free-dim) size
LN_EPS = 1e-5


# --------------------------------------------------------------------------
# host-side weight prep
# --------------------------------------------------------------------------

def _prep(inp):
    low_w_ih = inp["low_w_ih"]; low_w_hh = inp["low_w_hh"]
    high_w_ih = inp["high_w_ih"]; high_w_hh = inp["high_w_hh"]
    out_w = inp["out_w"]; ln_g = inp["ln_g"]; ln_b = inp["ln_b"]

    wx = low_w_ih[:, 0:256]; wh = low_w_ih[:, 256:512]; wl = low_w_ih[:, 512:768]
    a_hi = high_w_ih[:, 0:256]; b_hi = high_w_ih[:, 256:512]

    pieces = {}
    pieces["pw"] = (inp["proj_w"].T, 4)                     # [512,256] 4 k-chunks
    pieces["wx"] = (wx.T, 2)                                 # [256,768]
    pieces["wh"] = (wh.T, 2)
    pieces["wlrz"] = ((wl[0:512] + low_w_hh[0:512]).T, 2)    # [256,512]
    pieces["wln"] = (wl[512:768].T, 2)                       # [256,256]
    pieces["whhn"] = (low_w_hh[512:768].T, 2)
    pieces["aT"] = (a_hi.T, 2)                               # [256,768]
    pieces["brz"] = ((b_hi[0:512] + high_w_hh[0:512]).T, 2)  # [256,512]
    pieces["bn"] = (b_hi[512:768].T, 2)
    pieces["hhhn"] = (high_w_hh[512:768].T, 2)
    wg = out_w * ln_g[None, :]                               # [2,256]
    pieces["wg"] = (wg.T, 2)                                 # [256,2]

    cols = []
    offs = {}
    pos = 0
    for name, (mat, kchunks) in pieces.items():
        K, M = mat.shape
        assert K == kchunks * 128
        offs[name] = (pos, M)
        for k in range(kchunks):
            cols.append(mat[k * 128:(k + 1) * 128, :])
        pos += kchunks * M
    # identity block for I-add matmuls
    offs["ident"] = (pos, 128)
    cols.append(np.eye(128, dtype=np.float32)); pos += 128
    # ones/256 columns for LN mean reductions (2 k-chunks of [128,1])
    offs["ones"] = (pos, 1)
    cols.append(np.full((128, 1), 1.0 / 256.0, np.float32))
    cols.append(np.full((128, 1), 1.0 / 256.0, np.float32)); pos += 2
    # [1,2] ones row for partition broadcast (row0 only matters)
    offs["ones2"] = (pos, 2)
    cols.append(np.ones((128, 2), np.float32)); pos += 2
    wcat = np.concatenate(cols, axis=1).astype(np.float32)

    # biases -------------------------------------------------------------
    b_comb_low = inp["low_b_ih"] + np.concatenate(
        [inp["low_b_hh"][0:512], np.zeros(256, np.float32)])
    b_comb_high = inp["high_b_ih"] + np.concatenate(
        [inp["high_b_hh"][0:512], np.zeros(256, np.float32)])
    s1 = wg.sum(axis=1)                                      # [2]
    c0 = out_w @ ln_b + inp["out_b"]                         # [2]

    bias = np.zeros((128, 21), np.float32)
    for c in range(6):
        bias[:, c] = b_comb_low[c * 128:(c + 1) * 128]
    for c in range(2):
        bias[:, 6 + c] = inp["low_b_hh"][512 + c * 128: 512 + (c + 1) * 128]
    for c in range(2):
        bias[:, 8 + c] = b_comb_high[c * 128:(c + 1) * 128]
    for c in range(2):
        bias[:, 10 + c] = -b_comb_high[256 + c * 128: 256 + (c + 1) * 128]
    for c in range(2):
        bias[:, 12 + c] = b_comb_high[512 + c * 128: 512 + (c + 1) * 128]
    for c in range(2):
        bias[:, 14 + c] = inp["high_b_hh"][512 + c * 128: 512 + (c + 1) * 128]
    for c in range(2):
        bias[:, 16 + c] = inp["proj_b"][c * 128:(c + 1) * 128]
    bias[0, 18] = LN_EPS
    bias[0:2, 19] = c0
    bias[0:2, 20] = -s1
    return wcat, bias, offs


# --------------------------------------------------------------------------
# device module
# --------------------------------------------------------------------------

def _split_excess_waits(nc, max_waits=1):
    for bb in nc.main_func.blocks:
        nl = []
        for ins in bb.instructions:
            si = ins.sync_info
            if si is not None and si.on_wait and len(si.on_wait) > max_waits:
                w = list(si.on_wait)
                ex, keep = w[max_waits:], w[:max_waits]
                for k, ww in enumerate(ex):
                    stub = mybir.InstEventSemaphore(
                        name=f"{ins.name}-ws{k}", engine=ins.engine,
                        ins=[], outs=[],
                        sync_info=br.SyncInfo(on_wait=[ww], on_update=[]))
                    nc.register_instruction(stub)
                    nl.append(stub)
                ins.sync_info = br.SyncInfo(on_wait=keep,
                                            on_update=list(si.on_update))
            nl.append(ins)
        bb.instructions[:] = nl


def build_module(wcat_cols, n_tiles):
    nc = bass.Bass()
    x_d = nc.declare_dram_parameter("x", [BC, IN_DIM], F32, isOutput=False)
    w_d = nc.declare_dram_parameter("wcat", [128, wcat_cols], F32, isOutput=False)
    bias_d = nc.declare_dram_parameter("bias", [128, 21], F32, isOutput=False)
    zh_d = nc.declare_dram_parameter("zh_o", [BC, HH], F32, isOutput=True)
    zl_d = nc.declare_dram_parameter("zl_o", [BC, HL], F32, isOutput=True)
    lg_d = nc.declare_dram_parameter("lg_o", [BC, 2], F32, isOutput=True)
    return nc, (x_d, w_d, bias_d, zh_d, zl_d, lg_d)


def emit_kernel(nc, params, offs, n_tiles, reps=1):
    x_d, w_d, bias_d, zh_d, zl_d, lg_d = params
    wcat_cols = w_d.shape[1]

    from contextlib import ExitStack
    ctx = ExitStack()
    tc = ctx.enter_context(tile.TileContext(nc))
    wpool = ctx.enter_context(tc.tile_pool(name="w", bufs=1))
    apool = ctx.enter_context(tc.tile_pool(name="act", bufs=1))
    pspool = ctx.enter_context(tc.tile_pool(name="ps", bufs=1, space="PSUM"))

    # ---- weights: staged DMA + cast to f32r --------------------------------
    wr = wpool.tile([128, wcat_cols], F32R, tag="wr", name="wr")
    nchunk = 8
    cw = ((wcat_cols + nchunk - 1) // nchunk + 3) & ~3
    for i in range(nchunk):
        c0 = i * cw
        c1 = min(wcat_cols, c0 + cw)
        if c0 >= c1:
            break
        wtmp = wpool.tile([128, cw], F32, tag="wtmp", name=f"wtmp{i}", bufs=1)
        nc.sync.dma_start(out=wtmp[:, 0:c1 - c0], in_=w_d[:, c0:c1])
        eng = nc.vector if i % 2 == 0 else nc.gpsimd
        eng.tensor_copy(wr[:, c0:c1], wtmp[:, 0:c1 - c0])
    bias_sb = wpool.tile([128, 21], F32, tag="bias", name="bias")
    nc.sync.dma_start(out=bias_sb[:], in_=bias_d[:, :])
    ident = wpool.tile([128, 128], F32, tag="ident", name="ident")
    make_identity(nc, ident[:])

    def W(name, k, m0, mw=128):
        off, M = offs[name]
        return wr[:, off + k * M + m0: off + k * M + m0 + mw]

    identr = wr[:, offs["ident"][0]: offs["ident"][0] + 128]

    _BUFS = {"xnat": 2, "xT": 1, "xe": 2, "ax": 1, "ac": 1, "zl": 2, "zh": 2,
             "r": 2, "zc": 2, "t": 2, "u": 2, "n": 2, "d": 1, "e": 1,
             "stg": 1, "zsq": 1}
    _seq = [0]

    def PS(tag, shape=(128, TN), dt=F32):
        _seq[0] += 1
        return pspool.tile(list(shape), dt, tag=tag, name=f"{tag}_{_seq[0]}",
                           bufs=1)

    def act(t, shape=(128, TN), dt=F32):
        _seq[0] += 1
        base = t.rstrip("0123456789")
        return apool.tile(list(shape), dt, tag=t, name=f"{t}_{_seq[0]}",
                          bufs=_BUFS.get(base, 1))

    bias_ap = lambda c: bias_sb[:, c:c + 1]

    # ---- per-step bodies ---------------------------------------------------
    def low_step(zl, acomb):
        """one low-GRU step; zl: [2] f32r tiles, acomb: [6] f32r tiles."""
        rz_ps = []
        for m in range(4):
            ps = PS(f"P{m}")
            nc.tensor.matmul(ps[:], W("wlrz", 0, m * 128), zl[0][:], start=True, stop=False)
            nc.tensor.matmul(ps[:], W("wlrz", 1, m * 128), zl[1][:], start=False, stop=False)
            nc.tensor.matmul(ps[:], identr, acomb[m][:], start=False, stop=True)
            rz_ps.append(ps)
        r = []
        zc = []
        for c in range(2):
            rt = act(f"r{c}")
            nc.scalar.activation(out=rt[:], in_=rz_ps[c][:], func=AF.Sigmoid)
            r.append(rt)
        for c in range(2):
            zt = act(f"zc{c}")
            nc.scalar.activation(out=zt[:], in_=rz_ps[2 + c][:], func=AF.Sigmoid, scale=-1.0)
            zc.append(zt)
        gin_ps = []
        for m in range(2):
            ps = PS(f"P{4 + m}")
            nc.tensor.matmul(ps[:], W("wln", 0, m * 128), zl[0][:], start=True, stop=False)
            nc.tensor.matmul(ps[:], W("wln", 1, m * 128), zl[1][:], start=False, stop=False)
            nc.tensor.matmul(ps[:], identr, acomb[4 + m][:], start=False, stop=True)
            gin_ps.append(ps)
        ghn_ps = []
        for m in range(2):
            ps = PS(f"P{6 + m}")
            nc.tensor.matmul(ps[:], W("whhn", 0, m * 128), zl[0][:], start=True, stop=False)
            nc.tensor.matmul(ps[:], W("whhn", 1, m * 128), zl[1][:], start=False, stop=True)
            ghn_ps.append(ps)
        zl_new = []
        for c in range(2):
            tt = act(f"t{c}")
            nc.vector.scalar_tensor_tensor(out=tt[:], in0=ghn_ps[c][:], scalar=bias_ap(6 + c),
                                           in1=r[c][:], op0=AluOpType.add, op1=AluOpType.mult)
            uu = act(f"u{c}")
            nc.vector.tensor_tensor(out=uu[:], in0=tt[:], in1=gin_ps[c][:], op=AluOpType.add)
            nn = act(f"n{c}")
            nc.scalar.activation(out=nn[:], in_=uu[:], func=AF.Tanh)
            dd = act(f"d{c}")
            nc.gpsimd.tensor_tensor(out=dd[:], in0=nn[:], in1=zl[c][:], op=AluOpType.subtract)
            ee = act(f"e{c}")
            nc.gpsimd.tensor_tensor(out=ee[:], in0=zc[c][:], in1=dd[:], op=AluOpType.mult)
            zn = act(f"zl{c}", dt=F32R)
            nc.vector.tensor_tensor(out=zn[:], in0=ee[:], in1=zl[c][:], op=AluOpType.add)
            zl_new.append(zn)
        return zl_new

    def step0(ax):
        """first low step: z_l = z_h = 0, gates come from ax alone."""
        zl1 = []
        r = []
        zc = []
        for c in range(2):
            rt = act(f"r{c}")
            nc.scalar.activation(out=rt[:], in_=ax[c][:], func=AF.Sigmoid)
            r.append(rt)
        for c in range(2):
            zt = act(f"zc{c}")
            nc.scalar.activation(out=zt[:], in_=ax[2 + c][:], func=AF.Sigmoid, scale=-1.0)
            zc.append(zt)
        for c in range(2):
            tt = act(f"t{c}")
            nc.vector.tensor_scalar_mul(tt[:], r[c][:], bias_ap(6 + c))
            uu = act(f"u{c}")
            nc.vector.tensor_tensor(out=uu[:], in0=tt[:], in1=ax[4 + c][:], op=AluOpType.add)
            nn = act(f"n{c}")
            nc.scalar.activation(out=nn[:], in_=uu[:], func=AF.Tanh)
            zn = act(f"zl{c}", dt=F32R)
            nc.vector.tensor_tensor(out=zn[:], in0=zc[c][:], in1=nn[:], op=AluOpType.mult)
            zl1.append(zn)
        return zl1

    def high_step(zl, zh):
        """one high-GRU step; zh may be None (first high step, z_h = 0)."""
        rz_ps = []
        for m in range(4):
            ps = PS(f"P{m}")
            nc.tensor.matmul(ps[:], W("aT", 0, m * 128), zl[0][:], start=True, stop=False)
            nc.tensor.matmul(ps[:], W("aT", 1, m * 128), zl[1][:], start=False, stop=(zh is None))
            if zh is not None:
                nc.tensor.matmul(ps[:], W("brz", 0, m * 128), zh[0][:], start=False, stop=False)
                nc.tensor.matmul(ps[:], W("brz", 1, m * 128), zh[1][:], start=False, stop=True)
            rz_ps.append(ps)
        r = []
        zc = []
        for c in range(2):
            rt = act(f"r{c}")
            nc.scalar.activation(out=rt[:], in_=rz_ps[c][:], func=AF.Sigmoid, bias=bias_ap(8 + c))
            r.append(rt)
        for c in range(2):
            zt = act(f"zc{c}")
            nc.scalar.activation(out=zt[:], in_=rz_ps[2 + c][:], func=AF.Sigmoid,
                                 scale=-1.0, bias=bias_ap(10 + c))
            zc.append(zt)
        gin_ps = []
        for m in range(2):
            ps = PS(f"P{4 + m}")
            nc.tensor.matmul(ps[:], W("aT", 0, 512 + m * 128), zl[0][:], start=True, stop=False)
            nc.tensor.matmul(ps[:], W("aT", 1, 512 + m * 128), zl[1][:], start=False, stop=(zh is None))
            if zh is not None:
                nc.tensor.matmul(ps[:], W("bn", 0, m * 128), zh[0][:], start=False, stop=False)
                nc.tensor.matmul(ps[:], W("bn", 1, m * 128), zh[1][:], start=False, stop=True)
            gin_ps.append(ps)
        ghn_ps = []
        if zh is not None:
            for m in range(2):
                ps = PS(f"P{6 + m}")
                nc.tensor.matmul(ps[:], W("hhhn", 0, m * 128), zh[0][:], start=True, stop=False)
                nc.tensor.matmul(ps[:], W("hhhn", 1, m * 128), zh[1][:], start=False, stop=True)
                ghn_ps.append(ps)
        zh_new = []
        for c in range(2):
            tt = act(f"t{c}")
            if zh is not None:
                nc.vector.scalar_tensor_tensor(out=tt[:], in0=ghn_ps[c][:], scalar=bias_ap(14 + c),
                                               in1=r[c][:], op0=AluOpType.add, op1=AluOpType.mult)
            else:
                nc.vector.tensor_scalar_mul(tt[:], r[c][:], bias_ap(14 + c))
            uu = act(f"u{c}")
            nc.vector.tensor_tensor(out=uu[:], in0=tt[:], in1=gin_ps[c][:], op=AluOpType.add)
            nn = act(f"n{c}")
            nc.scalar.activation(out=nn[:], in_=uu[:], func=AF.Tanh, bias=bias_ap(12 + c))
            zn = act(f"zh{c}", dt=F32R)
            if zh is not None:
                dd = act(f"d{c}")
                nc.gpsimd.tensor_tensor(out=dd[:], in0=nn[:], in1=zh[c][:], op=AluOpType.subtract)
                ee = act(f"e{c}")
                nc.gpsimd.tensor_tensor(out=ee[:], in0=zc[c][:], in1=dd[:], op=AluOpType.mult)
                nc.vector.tensor_tensor(out=zn[:], in0=ee[:], in1=zh[c][:], op=AluOpType.add)
            else:
                nc.vector.tensor_tensor(out=zn[:], in0=zc[c][:], in1=nn[:], op=AluOpType.mult)
            zh_new.append(zn)
        return zh_new

    # ---- main tile loop ----------------------------------------------------
    rep_ctx = tc.For_i(0, reps, 1) if reps > 1 else None
    if rep_ctx is not None:
        rep_ctx.__enter__()
    for t in range(n_tiles):
        r0 = t * TN
        # input DMA: 4 natural tiles [128 batch rows, 512 features]
        xnat = []
        for c in range(4):
            xt = act(f"xnat{c}")
            nc.sync.dma_start(out=xt[:], in_=x_d[r0 + c * 128: r0 + (c + 1) * 128, :])
            xnat.append(xt)
        # transpose x -> feature-major xT [4 x (128 feat, 512 batch)]
        xT = [act(f"xT{fb}", dt=F32R) for fb in range(4)]
        for fb in range(4):
            for cb in range(4):
                tp = PS(f"P{(fb * 4 + cb) % 8}", (128, 128))
                nc.tensor.transpose(tp[:], xnat[cb][:, fb * 128:(fb + 1) * 128], ident[:])
                nc.scalar.activation(out=xT[fb][:, cb * 128:(cb + 1) * 128], in_=tp[:],
                                     func=AF.Identity)
        # projection + exact GELU -> x_embed [2 x (128, TN)] f32r
        xe = []
        for m in range(2):
            ps = PS(f"P{m}")
            for k in range(4):
                nc.tensor.matmul(ps[:], W("pw", k, m * 128), xT[k][:],
                                 start=(k == 0), stop=(k == 3))
            xem = act(f"xe{m}", dt=F32R)
            nc.scalar.activation(out=xem[:], in_=ps[:], func=AF.Gelu, bias=bias_ap(16 + m))
            xe.append(xem)
        # ax = x_embed @ Wx.T + b_comb_low  [6 x (128, TN)] f32r
        ax = []
        for m in range(6):
            ps = PS(f"P{m % 6}")
            nc.tensor.matmul(ps[:], W("wx", 0, m * 128), xe[0][:], start=True, stop=False)
            nc.tensor.matmul(ps[:], W("wx", 1, m * 128), xe[1][:], start=False, stop=True)
            am = act(f"ax{m}", dt=F32R)
            nc.scalar.activation(out=am[:], in_=ps[:], func=AF.Identity, bias=bias_ap(m))
            ax.append(am)

        # ---- recurrence ----
        zl = step0(ax)
        for _ in range(3):
            zl = low_step(zl, ax)
        zh = high_step(zl, None)

        for seg in range(2):
            acomb = []
            for m in range(6):
                ps = PS(f"P{m % 6}")
                nc.tensor.matmul(ps[:], W("wh", 0, m * 128), zh[0][:], start=True, stop=False)
                nc.tensor.matmul(ps[:], W("wh", 1, m * 128), zh[1][:], start=False, stop=False)
                nc.tensor.matmul(ps[:], identr, ax[m][:], start=False, stop=True)
                am = act(f"ac{m}", dt=F32R)
                nc.scalar.activation(out=am[:], in_=ps[:], func=AF.Identity)
                acomb.append(am)
            for _ in range(4):
                zl = low_step(zl, acomb)
            zh = high_step(zl, zh)

        # ---- output head: LayerNorm + tiny linear ----
        zsq = []
        for c in range(2):
            zq = act(f"zsq{c}", dt=F32R)
            nc.scalar.activation(out=zq[:], in_=zh[c][:], func=AF.Square)
            zsq.append(zq)
        mu_ps = PS("P0", (1, TN))
        nc.tensor.matmul(mu_ps[:], W("ones", 0, 0, 1), zh[0][:], start=True, stop=False)
        nc.tensor.matmul(mu_ps[:], W("ones", 1, 0, 1), zh[1][:], start=False, stop=True)
        ex2_ps = PS("P1", (1, TN))
        nc.tensor.matmul(ex2_ps[:], W("ones", 0, 0, 1), zsq[0][:], start=True, stop=False)
        nc.tensor.matmul(ex2_ps[:], W("ones", 1, 0, 1), zsq[1][:], start=False, stop=True)
        wgz_ps = PS("P2", (2, TN))
        nc.tensor.matmul(wgz_ps[:], W("wg", 0, 0, 2), zh[0][:], start=True, stop=False)
        nc.tensor.matmul(wgz_ps[:], W("wg", 1, 0, 2), zh[1][:], start=False, stop=True)
        mu_sb = act("mu", (1, TN), F32R)
        nc.scalar.activation(out=mu_sb[:], in_=mu_ps[:], func=AF.Identity)
        musq = act("musq", (1, TN))
        nc.scalar.activation(out=musq[:], in_=mu_ps[:], func=AF.Square)
        var = act("var", (1, TN))
        nc.vector.tensor_tensor(out=var[:], in0=ex2_ps[:], in1=musq[:], op=AluOpType.subtract)
        sd = act("sd", (1, TN))
        nc.scalar.activation(out=sd[:], in_=var[:], func=AF.Sqrt, bias=bias_sb[0:1, 18:19])
        rstd = act("rstd", (1, TN), F32R)
        with nc.allow_low_precision(reason="f32r is fp32-width storage"):
            nc.vector.reciprocal(rstd[:], sd[:])
        v = act("v", (1, TN), F32R)
        nc.vector.tensor_tensor(out=v[:], in0=mu_sb[:], in1=rstd[:], op=AluOpType.mult)
        rb2_ps = PS("P3", (2, TN))
        nc.tensor.matmul(rb2_ps[:], wr[0:1, offs["ones2"][0]: offs["ones2"][0] + 2],
                         rstd[:], start=True, stop=True)
        v2_ps = PS("P4", (2, TN))
        nc.tensor.matmul(v2_ps[:], wr[0:1, offs["ones2"][0]: offs["ones2"][0] + 2],
                         v[:], start=True, stop=True)
        rb2 = act("rb2", (2, TN))
        nc.scalar.activation(out=rb2[:], in_=rb2_ps[:], func=AF.Identity)
        u2 = act("u2", (2, TN))
        nc.vector.tensor_tensor(out=u2[:], in0=wgz_ps[:], in1=rb2[:], op=AluOpType.mult)
        w2 = act("w2", (2, TN))
        nc.vector.scalar_tensor_tensor(out=w2[:], in0=v2_ps[:], scalar=bias_sb[0:2, 20:21],
                                       in1=u2[:], op0=AluOpType.mult, op1=AluOpType.add)
        lg = act("lg", (2, TN))
        nc.vector.tensor_scalar_add(lg[:], w2[:], bias_sb[0:2, 19:20])
        for j in range(2):
            nc.sync.dma_start(out=lg_d[r0:r0 + TN, j:j + 1].rearrange("a b -> b a"),
                              in_=lg[j:j + 1, :])

        # ---- transpose states back to batch-major and store ----
        for si, (state, dram) in enumerate([(zh, zh_d), (zl, zl_d)]):
            stg = [act(f"stg{si}{cb}", (128, 256)) for cb in range(4)]
            for f in range(2):
                for cb in range(4):
                    tp = PS(f"P{(f * 4 + cb) % 8}", (128, 128), F32R)
                    nc.tensor.transpose(tp[:], state[f][:, cb * 128:(cb + 1) * 128], identr)
                    nc.scalar.activation(out=stg[cb][:, f * 128:(f + 1) * 128],
                                         in_=tp[:], func=AF.Identity)
            for cb in range(4):
                nc.sync.dma_start(out=dram[r0 + cb * 128: r0 + (cb + 1) * 128, :],
                                  in_=stg[cb][:])

    if rep_ctx is not None:
        rep_ctx.__exit__(None, None, None)
    ctx.close()
    return nc


def build(n_tiles=BC // TN):
    # wcat column count must match _prep; compute from a dummy
    dummy = {k: np.zeros(s, np.float32) for k, s in [
        ("proj_w", (256, 512)), ("proj_b", (256,)),
        ("low_w_ih", (768, 768)), ("low_w_hh", (768, 256)),
        ("low_b_ih", (768,)), ("low_b_hh", (768,)),
        ("high_w_ih", (768, 512)), ("high_w_hh", (768, 256)),
        ("high_b_ih", (768,)), ("high_b_hh", (768,)),
        ("ln_g", (256,)), ("ln_b", (256,)), ("out_w", (2, 256)), ("out_b", (2,))]}
    wcat, _, offs = _prep(dummy)
    nc, params = build_module(wcat.shape[1], n_tiles)
    emit_kernel(nc, params, offs, n_tiles)
    _split_excess_waits(nc)
    return nc


_CACHED = {}


def kernel(trace=False, n_tiles=BC // TN, reps=1, **inputs):
    inputs = {k: np.asarray(v, np.float32) for k, v in inputs.items()}
    wcat, bias, offs = _prep(inputs)
    key = (n_tiles, reps)
    if key not in _CACHED:
        nc, params = build_module(wcat.shape[1], n_tiles)
        emit_kernel(nc, params, offs, n_tiles, reps)
        _split_excess_waits(nc)
        _CACHED[key] = nc
    nc = _CACHED[key]

    x = inputs["x"]
    in_maps = []
    for c in range(N_CORES):
        in_maps.append({
            "x": np.ascontiguousarray(x[c * BC:(c + 1) * BC]),
            "wcat": wcat, "bias": bias,
        })
    res = run_bass_kernel_spmd(nc, in_maps, core_ids=list(range(N_CORES)),
                               trace=trace)
    rows = n_tiles * TN
    zh = np.empty((B, HH), np.float32)
    zl = np.empty((B, HL), np.float32)
    lg = np.empty((B, 2), np.float32)
    for c in range(N_CORES):
        zh[c * BC: c * BC + rows] = res.results[c]["zh_o"][:rows]
        zl[c * BC: c * BC + rows] = res.results[c]["zl_o"][:rows]
        lg[c * BC: c * BC + rows] = res.results[c]["lg_o"][:rows]
    kernel.last_results = res
    return zh, zl, lg
